# revision 6
# baseline (speedup 1.0000x reference)
"""LocallyConnected2d (B=8, C_in=32, 48x48, C_out=32, 3x3, pad 1) on 8 trn2 cores.

Strategy: shard the spatial-location axis L = H*W across cores (6 image rows
each). Per location l the op is an (8x288)@(288x32) GEMM with location-unique
weights; weight streaming (42.5 MB fp16 total) dominates -> memory-bound.

v2 design (vs v1 baseline at 63 us):
  - Bias is folded into the matmul as a 97th contraction row: x gets a
    constant-1.0 partition row 96, each weight tile gets bias[l, o] in row 96
    of its kh=0 block. This deletes all 72 bias matmuls and the one-hot /
    bias-table machinery (PE ingest drops 25%). K=96 and K=97 both round to
    the same (128, 32) PE tiling mode, so no mode switches.
  - ALL weight tiles are preloaded: 9 tiles x [97, 6144B-rows] issued
    back-to-back on the gpsimd queue at t=0 (SBUF easily holds the 5.4 MB),
    so the 16 DMA engines stream weights continuously instead of stalling
    between tiles. 6144B descriptors measured ~19.4 GB/s/engine vs 15.1 for
    9216B. x/stat goes first on the sync queue; output stores also on sync.
  - Per weight tile (32 locations = 2 groups) matmuls accumulate in a
    2-bank PSUM tile [128, 1024]; 4 copies (one per PE column group j)
    move ps[32j:32j+8] for both groups to SBUF as fp16, split across the
    vector and scalar engines. Output is [32, 2304] fp16 (4x less store
    traffic than fp32 [128, *]), unscrambled to NCHW fp32 on the host.
  - Contraction (d=288) is split into 3 kh-rounds of K=96=(3 kw x 32 c),
    PSUM-accumulated; x halo slice lives in SBUF replicated 3x with kw
    column shifts, so any patch is a plain strided AP slice.
"""

import numpy as np

import concourse.bacc as bacc
import concourse.tile as tile
from concourse import mybir
from concourse.bass_utils import run_bass_kernel_spmd

B, C_IN, H, W = 8, 32, 48, 48
C_OUT = 32
N_CORES = 8
RP = H // N_CORES  # rows per core (6)
LP = RP * W  # locations per core (288)
NGRP = LP // 16  # 16-loc output groups per core (18)

DT16 = True  # fp16 operand path (halves weight traffic)
DT = mybir.dt.float16 if DT16 else mybir.dt.float32
NPDT = np.float16 if DT16 else np.float32
F32 = mybir.dt.float32

TL = 32  # locations per weight tile -> [97, 6144B] DMAs
NT = LP // TL  # 9 weight tiles
XF = (RP + 2) * W * B  # x3 free size (3072)

_nc = None


def _build():
    nc = bacc.Bacc(
        "TRN2", target_bir_lowering=False, debug=False, num_devices=N_CORES
    )
    stat = nc.dram_tensor("stat", [97, XF], DT, kind="ExternalInput")
    wds = [
        nc.dram_tensor(f"w{t}", [97, 3 * TL * C_OUT], DT, kind="ExternalInput")
        for t in range(NT)
    ]
    out = nc.dram_tensor("out", [32, NGRP * 128], DT, kind="ExternalOutput")

    with tile.TileContext(nc) as tc:
        with (
            tc.tile_pool(name="xpool", bufs=1) as xpool,
            tc.tile_pool(name="wpool", bufs=NT) as wpool,
            tc.tile_pool(name="opool", bufs=1) as opool,
            tc.tile_pool(name="pspool", bufs=4, space="PSUM") as pspool,
        ):
            stat_sb = xpool.tile([97, XF], DT, tag="stat")
            nc.sync.dma_start(stat_sb[:, :], stat[:, :])

            wts = []
            for t in range(NT):
                wt = wpool.tile([97, 3 * TL * C_OUT], DT, tag="wt")
                nc.gpsimd.dma_start(wt[:, :], wds[t][:, :])
                wts.append(wt)

            out_sb = opool.tile([128, NGRP * 128], DT)

            for t in range(NT):
                ps = pspool.tile([128, 1024], F32)
                for gl in range(2):
                    gi = 2 * t + gl
                    rl, qg = divmod(gi, 3)
                    for m in range(4):
                        for kh in range(3):
                            kd = 97 if kh == 0 else 96
                            for j in range(4):
                                q = qg * 16 + m * 4 + j
                                ll = gl * 16 + m * 4 + j
                                off = ((rl + kh) * W + q) * B
                                nc.tensor.matmul(
                                    ps[
                                        32 * j : 32 * j + B,
                                        gl * 512 + m * 32 : gl * 512 + (m + 1) * 32,
                                    ],
                                    stat_sb[0:kd, off : off + B],
                                    wts[t][
                                        0:kd, (kh * TL + ll) * 32 : (kh * TL + ll + 1) * 32
                                    ],
                                    start=(kh == 0),
                                    stop=(kh == 2),
                                    skip_group_check=True,
                                    tile_position=(0, 32 * j),
                                )
                # copy both groups' (j, b) rows to SBUF as fp16; split j over
                # vector and scalar so copies never serialize the stream
                for j in range(4):
                    src = ps[32 * j : 32 * j + 8, 0:1024].rearrange(
                        "p (g f) -> p g f", g=2
                    )[:, :, 0:128]
                    dst = out_sb[
                        32 * j : 32 * j + 8, 256 * t : 256 * t + 256
                    ].rearrange("p (g f) -> p g f", g=2)
                    if j < 2:
                        nc.vector.tensor_copy(dst, src)
                    else:
                        nc.scalar.copy(dst, src)
                if t % 3 == 2:
                    s = t // 3
                    for j in range(4):
                        nc.sync.dma_start(
                            out[8 * j : 8 * j + 8, 768 * s : 768 * (s + 1)],
                            out_sb[32 * j : 32 * j + 8, 768 * s : 768 * (s + 1)],
                        )
    nc.compile()
    return nc


def _shard(inputs):
    x = np.asarray(inputs["x"], np.float32)
    weight = np.asarray(inputs["weight"], np.float32)[0]  # (288, L, 32)
    bias = np.asarray(inputs["bias"], np.float32)[0]  # (32, 48, 48)
    xp = np.pad(x, ((0, 0), (0, 0), (1, 1), (1, 1)))  # (b, c, 50, 50)
    bias_t = bias.reshape(C_OUT, H * W).T  # (L, C_OUT)
    wflat = weight.reshape(C_IN, 3, 3, H * W, C_OUT)  # (c, kh, kw, l, o)

    in_maps = []
    for k in range(N_CORES):
        r0 = RP * k
        l0 = LP * k

        x3h = np.empty((3, C_IN, RP + 2, W, B), np.float32)
        for kw in range(3):
            x3h[kw] = xp[:, :, r0 : r0 + RP + 2, kw : kw + W].transpose(1, 2, 3, 0)
        stat = np.empty((97, XF), NPDT)
        stat[0:96] = x3h.reshape(96, XF).astype(NPDT)
        stat[96] = 1.0

        # W tiles: [(kw c) + bias row, (kh, lg, o)]
        wk = wflat[:, :, :, l0 : l0 + LP, :]  # (c, kh, kw, LP, o)
        wall = wk.transpose(2, 0, 1, 3, 4).reshape(96, 3, LP, C_OUT)
        m = {"stat": stat}
        for t in range(NT):
            wt = np.zeros((97, 3 * TL * C_OUT), NPDT)
            wt[0:96] = (
                wall[:, :, TL * t : TL * (t + 1), :]
                .reshape(96, 3 * TL * C_OUT)
                .astype(NPDT)
            )
            wt[96, 0 : TL * C_OUT] = (
                bias_t[l0 + TL * t : l0 + TL * (t + 1), :].reshape(-1).astype(NPDT)
            )
            m[f"w{t}"] = wt
        in_maps.append(m)
    return in_maps


def _get_nc():
    global _nc
    if _nc is None:
        _nc = _build()
    return _nc


def _gather(results):
    # out row 8j+b holds y[b, o, r, q] at col gi*128 + m*32 + o,
    # with r = gi//3, q = (gi%3)*16 + m*4 + j
    y = np.empty((B, C_OUT, H, W), np.float32)
    for k in range(N_CORES):
        arr = results[k]["out"].astype(np.float32)
        arr = arr.reshape(4, B, NGRP, 4, C_OUT)  # (j, b, gi, m, o)
        arr = arr.transpose(1, 4, 2, 3, 0)  # (b, o, gi, m, j)
        arr = arr.reshape(B, C_OUT, RP, 3, 4, 4)  # (b, o, r, qg, m, j)
        y[:, :, RP * k : RP * (k + 1), :] = arr.reshape(B, C_OUT, RP, W)
    return y


def kernel(**inputs):
    nc = _get_nc()
    res = run_bass_kernel_spmd(nc, _shard(inputs), list(range(N_CORES)))
    return _gather(res.results)
